# revision 9
# baseline (speedup 1.0000x reference)
"""DeepseekV32 indexer kernel for 8x TRN2 NeuronCores (Bass/Tile).

Strategy: sequence-parallel over queries (256 q/core). Each core computes its
k-projection shard, LayerNorm+RoPE+bf16 FWHT, AllGathers kT across the 8
cores, then computes its queries' projections and per-head relu-weighted
scores against all 2048 keys. Top-k runs on host from exact fp32 scores.
"""

import numpy as np
import ml_dtypes

import concourse.bacc as bacc
import concourse.bass as bass
import concourse.mybir as mybir
import concourse.tile as tile
from concourse.bass_utils import run_bass_kernel_spmd
from concourse.masks import make_identity

bf16 = ml_dtypes.bfloat16
FP32 = mybir.dt.float32
BF16 = mybir.dt.bfloat16

B, S, H, D, DR, HID, QLR, TOPK = 1, 2048, 32, 128, 64, 4096, 1536, 1024
DN = D - DR
N_CORES = 8
SS = S // N_CORES            # 256 queries/keys per core shard
NST = SS // 128              # s-tiles per shard (2)
NKI_HID = HID // 128         # 32 contraction chunks for k/w proj
NKI_QLR = QLR // 128         # 12 contraction chunks for q proj
NQCH = (H * D) // 512        # 8 output column chunks of 512 for q proj
KB = 512                     # key block for score stage
FWHT_SCALE = float(bf16(D ** -0.5))  # bf16-rounded 128**-0.5, exact in fp32
W_SCALE1 = float(np.float32(H ** -0.5))
W_SCALE2 = float(np.float32(D ** -0.5))


def _emit_fwht(nc, pool, src, n_heads, tag):
    """Staged bf16 FWHT over trailing D=128 of [128, n_heads*128] bf16 tile.

    Matches the reference's per-stage bf16 rounding exactly. Returns the tile
    holding the scaled result (may be a scratch tile from `pool`).
    """
    fam = "q" if n_heads > 1 else "k"
    scratch = pool.tile([128, n_heads * 128], BF16, tag=f"fw_{fam}",
                        name=f"fw_scratch_{tag}")
    a_t, b_t = src, scratch
    hh = 1
    while hh < D:
        nb = D // (2 * hh)
        sv = a_t[:].rearrange("p (H nb two i) -> p H nb two i", H=n_heads, nb=nb, two=2)
        dv = b_t[:].rearrange("p (H nb two i) -> p H nb two i", H=n_heads, nb=nb, two=2)
        a = sv[:, :, :, 0, :]
        b = sv[:, :, :, 1, :]
        nc.vector.tensor_add(dv[:, :, :, 0, :], a, b)
        nc.vector.tensor_sub(dv[:, :, :, 1, :], a, b)
        a_t, b_t = b_t, a_t
        hh *= 2
    nc.vector.tensor_scalar_mul(b_t[:], a_t[:], FWHT_SCALE)
    return b_t


def _emit_layernorm(nc, pool, kt):
    """In-place-ish LN over free dim of kt [128, D] fp32; returns normed tile."""
    red = pool.tile([128, 1], FP32, tag="ln_red")
    t = pool.tile([128, D], FP32, tag="ln_t")
    sq = pool.tile([128, D], FP32, tag="ln_sq")
    mu = pool.tile([128, 1], FP32, tag="ln_mu")
    veps = pool.tile([128, 1], FP32, tag="ln_veps")
    s0 = pool.tile([128, 1], FP32, tag="ln_s0")
    r0 = pool.tile([128, 1], FP32, tag="ln_r0")
    t1 = pool.tile([128, 1], FP32, tag="ln_t1")
    s1 = pool.tile([128, 1], FP32, tag="ln_s1")
    rinv = pool.tile([128, 1], FP32, tag="ln_rinv")
    kn = pool.tile([128, D], FP32, tag="ln_kn")

    nc.vector.tensor_reduce(red[:], kt[:], axis=mybir.AxisListType.X, op=mybir.AluOpType.add)
    nc.vector.tensor_scalar_mul(mu[:], red[:], 1.0 / D)
    nc.vector.tensor_scalar_sub(t[:], kt[:], mu[:])
    nc.vector.tensor_mul(sq[:], t[:], t[:])
    nc.vector.tensor_reduce(red[:], sq[:], axis=mybir.AxisListType.X, op=mybir.AluOpType.add)
    # var = red/D ; veps = var + 1e-5
    nc.vector.tensor_scalar(veps[:], red[:], 1.0 / D, 1e-5,
                            op0=mybir.AluOpType.mult, op1=mybir.AluOpType.add)
    nc.scalar.activation(s0[:], veps[:], mybir.ActivationFunctionType.Sqrt)
    # one Newton step: s1 = 0.5*(s0 + veps/s0), then rinv = 1/s1
    nc.vector.reciprocal(r0[:], s0[:])
    nc.vector.tensor_mul(t1[:], veps[:], r0[:])
    nc.vector.tensor_scalar(s1[:], t1[:], s0[:], 0.5,
                            op0=mybir.AluOpType.add, op1=mybir.AluOpType.mult)
    nc.vector.reciprocal(rinv[:], s1[:])
    nc.vector.tensor_scalar_mul(kn[:], t[:], rinv[:])
    # kn_gamma/kn_beta are ones/zeros per the problem spec: exact no-op.
    return kn


def _emit_rope(nc, pool, pe_f32, cos_t, sin_t, out_bf16_pe, n_heads, tag):
    """RoPE on pe_f32 [128, n_heads, DR] fp32 -> out_bf16_pe (same view, bf16).

    cos_t/sin_t: [128, DR//2] fp32 tiles. Broadcast over heads via stride-0 AP.
    Writes interleaved-pair rotated values, cast to bf16.
    """
    NP = DR // 2
    xv = pe_f32.rearrange("p h (i two) -> p h i two", two=2)
    x1 = xv[:, :, :, 0].transpose([0, 2, 1])   # [p, i, h]
    x2 = xv[:, :, :, 1].transpose([0, 2, 1])
    cos_b = cos_t[:].broadcast_to((128, NP, n_heads))
    sin_b = sin_t[:].broadcast_to((128, NP, n_heads))
    tv = lambda t: t[:].rearrange("p (i h) -> p i h", h=n_heads)
    ov = out_bf16_pe.rearrange("p h (i two) -> p h i two", two=2)
    o1 = ov[:, :, :, 0].transpose([0, 2, 1])
    o2 = ov[:, :, :, 1].transpose([0, 2, 1])
    t1 = pool.tile([128, NP * n_heads], FP32, tag="rope_a", name=f"rt1_{tag}")
    t2 = pool.tile([128, NP * n_heads], FP32, tag="rope_b", name=f"rt2_{tag}")
    nc.vector.tensor_mul(tv(t1), x1, cos_b)
    nc.vector.tensor_mul(tv(t2), x2, sin_b)
    nc.vector.tensor_sub(o1, tv(t1), tv(t2))
    t3 = pool.tile([128, NP * n_heads], FP32, tag="rope_a", name=f"rt3_{tag}")
    t4 = pool.tile([128, NP * n_heads], FP32, tag="rope_b", name=f"rt4_{tag}")
    nc.vector.tensor_mul(tv(t3), x1, sin_b)
    nc.vector.tensor_mul(tv(t4), x2, cos_b)
    nc.vector.tensor_add(o2, tv(t3), tv(t4))


def build_nc(n_cores=N_CORES):
    """Build + compile the SPMD program (identical on all cores)."""
    nc = bacc.Bacc("TRN2", target_bir_lowering=False, debug=False,
                   num_devices=n_cores)
    s_keys = SS * n_cores  # total keys after gather

    xT = nc.dram_tensor("xT", [HID, SS], FP32, kind="ExternalInput")
    qrT = nc.dram_tensor("qrT", [QLR, SS], FP32, kind="ExternalInput")
    wqT = nc.dram_tensor("wqT", [QLR, H * D], FP32, kind="ExternalInput")
    wkwT = nc.dram_tensor("wkwT", [HID, D + H], FP32, kind="ExternalInput")
    cosd = nc.dram_tensor("cosd", [SS, DR // 2], FP32, kind="ExternalInput")
    sind = nc.dram_tensor("sind", [SS, DR // 2], FP32, kind="ExternalInput")
    scores = nc.dram_tensor("scores", [SS, s_keys], FP32, kind="ExternalOutput")

    with tile.TileContext(nc) as tc:
        _build_body(tc, nc, n_cores, s_keys,
                    xT.ap(), qrT.ap(), wqT.ap(), wkwT.ap(),
                    cosd.ap(), sind.ap(), scores.ap())
    nc.compile()
    return nc


def _build_body(tc, nc, n_cores, s_keys, xT, qrT, wqT, wkwT, cosd, sind, scores):
    import contextlib
    ctx = contextlib.ExitStack()
    with ctx:
        const = ctx.enter_context(tc.tile_pool(name="const", bufs=1))
        persist = ctx.enter_context(tc.tile_pool(name="persist", bufs=1))
        work = ctx.enter_context(tc.tile_pool(name="work", bufs=2))
        small = ctx.enter_context(tc.tile_pool(name="small", bufs=2))
        stream = ctx.enter_context(tc.tile_pool(name="stream", bufs=4))
        relu_p = ctx.enter_context(tc.tile_pool(name="relu", bufs=3))
        rope_p = ctx.enter_context(tc.tile_pool(name="rope", bufs=2))
        acc_p = ctx.enter_context(tc.tile_pool(name="accp", bufs=4))
        ps_q = ctx.enter_context(tc.tile_pool(name="ps_q", bufs=1, space="PSUM"))
        ps_kw = ctx.enter_context(tc.tile_pool(name="ps_kw", bufs=1, space="PSUM"))
        ps_t = ctx.enter_context(tc.tile_pool(name="ps_t", bufs=2, space="PSUM"))
        ps_s = ctx.enter_context(tc.tile_pool(name="ps_s", bufs=3, space="PSUM"))
        dram = ctx.enter_context(tc.tile_pool(name="dram", bufs=1, space="DRAM"))

        identity = const.tile([128, 128], BF16)
        make_identity(nc, identity[:])

        # ---- load activations (pre-transposed on host) ----
        xt_sb = persist.tile([128, NKI_HID, SS], FP32)
        nc.sync.dma_start(
            xt_sb[:],
            xT.rearrange("(ki p) s -> p ki s", p=128))
        qr_sb = persist.tile([128, NKI_QLR, SS], FP32)
        nc.sync.dma_start(
            qr_sb[:],
            qrT.rearrange("(ki p) s -> p ki s", p=128))
        kw_sb = persist.tile([128, NKI_HID, D + H], FP32)
        nc.sync.dma_start(
            kw_sb[:],
            wkwT.rearrange("(ki p) f -> p ki f", p=128))
        cos_sb = const.tile([128, NST, DR // 2], FP32)
        nc.sync.dma_start(cos_sb[:], cosd.rearrange("(st p) i -> p st i", p=128))
        sin_sb = const.tile([128, NST, DR // 2], FP32)
        nc.sync.dma_start(sin_sb[:], sind.rearrange("(st p) i -> p st i", p=128))

        # ---- k + w projection, LN, rope, fwht, transpose ----
        w_sb = persist.tile([128, NST, H], FP32)     # per-query head weights
        ktloc = persist.tile([128, NST * 128], BF16)  # kT for own shard [d, s]
        for st in range(NST):
            psum_kw = ps_kw.tile([128, D + H], FP32)
            for ki in range(NKI_HID):
                nc.tensor.matmul(psum_kw[:],
                                 xt_sb[:, ki, st * 128:(st + 1) * 128],
                                 kw_sb[:, ki, :],
                                 start=(ki == 0), stop=(ki == NKI_HID - 1))
            kproj = work.tile([128, D], FP32, tag="kproj")
            nc.scalar.activation(kproj[:], psum_kw[:, 0:D],
                                 mybir.ActivationFunctionType.Copy)
            nc.vector.tensor_scalar(w_sb[:, st, :], psum_kw[:, D:D + H],
                                    W_SCALE1, W_SCALE2,
                                    op0=mybir.AluOpType.mult,
                                    op1=mybir.AluOpType.mult)
            kn = _emit_layernorm(nc, small, kproj)
            kb_t = work.tile([128, D], BF16, tag="kb")
            nc.vector.tensor_copy(kb_t[:, 0:DN], kn[:, 0:DN])  # nope part, bf16 cast
            knv = kn[:].rearrange("p (h d) -> p h d", h=1)
            kbv = kb_t[:].rearrange("p (h d) -> p h d", h=1)
            _emit_rope(nc, rope_p, knv[:, :, DN:D], cos_sb[:, st, :], sin_sb[:, st, :],
                       kbv[:, :, DN:D], 1, "k")
            kfw = _emit_fwht(nc, work, kb_t, 1, "k")
            pst = ps_t.tile([128, 128], BF16)
            nc.tensor.transpose(pst[:], kfw[:], identity[:])
            nc.scalar.activation(ktloc[:, st * 128:(st + 1) * 128], pst[:],
                                 mybir.ActivationFunctionType.Copy)

        # ---- allgather kT across cores ----
        kT_sb = persist.tile([128, s_keys], BF16)
        if n_cores > 1:
            kt_in = dram.tile([128, SS], BF16)
            kt_out = dram.tile([n_cores, 128, SS], BF16, addr_space="Shared")
            nc.sync.dma_start(kt_in[:], ktloc[:])
            nc.gpsimd.collective_compute(
                "AllGather", mybir.AluOpType.bypass,
                replica_groups=[list(range(n_cores))],
                ins=[kt_in.opt()], outs=[kt_out.opt()])
            nc.sync.dma_start(
                kT_sb[:],
                kt_out[:].transpose([1, 0, 2]))
        else:
            nc.vector.tensor_copy(kT_sb[:], ktloc[:])

        # ---- q projection + rope + fwht + per-head transpose ----
        qT_sb = persist.tile([128, NST * H, 128], BF16)  # [(d), (st,h), q]
        q_bfs = [persist.tile([128, H * D], BF16, name=f"q_bf{st}")
                 for st in range(NST)]
        qpes = [persist.tile([128, H, DR], FP32, name=f"qpe{st}")
                for st in range(NST)]
        for nh in range(NQCH):
            psums = [ps_q.tile([128, 512], FP32, name=f"psum_q{st}")
                     for st in range(NST)]
            for ki in range(NKI_QLR):
                wq_t = stream.tile([128, 512], FP32, tag="wq_t")
                nc.sync.dma_start(wq_t[:],
                                  wqT[ki * 128:(ki + 1) * 128,
                                      nh * 512:(nh + 1) * 512])
                for st in range(NST):
                    nc.tensor.matmul(psums[st][:],
                                     qr_sb[:, ki, st * 128:(st + 1) * 128],
                                     wq_t[:],
                                     start=(ki == 0), stop=(ki == NKI_QLR - 1))
            for st in range(NST):
                pv = psums[st][:].rearrange("p (h d) -> p h d", d=D)
                qbv = q_bfs[st][:].rearrange("p (h d) -> p h d", d=D)
                nc.scalar.activation(qbv[:, nh * 4:(nh + 1) * 4, 0:DN],
                                     pv[:, :, 0:DN],
                                     mybir.ActivationFunctionType.Copy)
                nc.scalar.activation(qpes[st][:, nh * 4:(nh + 1) * 4, :],
                                     pv[:, :, DN:D],
                                     mybir.ActivationFunctionType.Copy)
        for st in range(NST):
            qbv = q_bfs[st][:].rearrange("p (h d) -> p h d", d=D)
            _emit_rope(nc, rope_p, qpes[st][:], cos_sb[:, st, :], sin_sb[:, st, :],
                       qbv[:, :, DN:D], H, f"q{st}")
            qfw = _emit_fwht(nc, work, q_bfs[st], H, f"q{st}")
            qfv = qfw[:].rearrange("p (h d) -> p h d", d=D)
            for h in range(H):
                pst = ps_t.tile([128, 128], BF16)
                nc.tensor.transpose(pst[:], qfv[:, h, :], identity[:])
                nc.scalar.activation(qT_sb[:, st * H + h, :], pst[:],
                                     mybir.ActivationFunctionType.Copy)

        # ---- scores: per (st, key-block, head) ----
        nkb = s_keys // KB if s_keys >= KB else 1
        kbw = min(KB, s_keys)
        for st in range(NST):
            for kb in range(nkb):
                acc = acc_p.tile([128, kbw], FP32, tag="acc", name=f"acc{st}_{kb}")
                for h in range(H):
                    pss = ps_s.tile([128, kbw], FP32)
                    nc.tensor.matmul(pss[:], qT_sb[:, st * H + h, :],
                                     kT_sb[:, kb * kbw:(kb + 1) * kbw],
                                     start=True, stop=True)
                    r_sb = relu_p.tile([128, kbw], FP32, tag="r_sb")
                    nc.scalar.activation(r_sb[:], pss[:],
                                         mybir.ActivationFunctionType.Relu)
                    if h == 0:
                        nc.vector.tensor_scalar_mul(acc[:], r_sb[:],
                                                    w_sb[:, st, h:h + 1])
                    else:
                        nc.vector.scalar_tensor_tensor(
                            acc[:], r_sb[:], w_sb[:, st, h:h + 1], acc[:],
                            op0=mybir.AluOpType.mult, op1=mybir.AluOpType.add)
                nc.sync.dma_start(
                    scores[st * 128:(st + 1) * 128, kb * kbw:(kb + 1) * kbw],
                    acc[:])


_NC_CACHE = {}


def _get_nc(n_cores=N_CORES):
    if n_cores not in _NC_CACHE:
        _NC_CACHE[n_cores] = build_nc(n_cores)
    return _NC_CACHE[n_cores]


def host_prep(x, q_resid, freqs, wq_b, wk, w_weights):
    """Host-side input prep shared by kernel() and tests."""
    x2 = np.ascontiguousarray(x.reshape(S, HID).astype(np.float32))
    qr2 = np.ascontiguousarray(q_resid.reshape(S, QLR).astype(np.float32))
    xT = x2.T                      # [HID, S] view
    qrT = qr2.T                    # [QLR, S] view
    wqT = np.ascontiguousarray(wq_b.astype(np.float32).T)     # [QLR, H*D]
    wkwT = np.ascontiguousarray(
        np.concatenate([wk.astype(np.float32).T,
                        w_weights.astype(np.float32).T], axis=1))  # [HID, D+H]
    cos = np.cos(freqs.astype(np.float32)).astype(np.float32)  # [S, DR//2]
    sin = np.sin(freqs.astype(np.float32)).astype(np.float32)
    return xT, qrT, wqT, wkwT, cos, sin


def make_in_maps(xT, qrT, wqT, wkwT, cos, sin, n_cores=N_CORES):
    maps = []
    for c in range(n_cores):
        sl = slice(c * SS, (c + 1) * SS)
        maps.append({
            "xT": np.ascontiguousarray(xT[:, sl]),
            "qrT": np.ascontiguousarray(qrT[:, sl]),
            "wqT": wqT,
            "wkwT": wkwT,
            "cosd": np.ascontiguousarray(cos[sl]),
            "sind": np.ascontiguousarray(sin[sl]),
        })
    return maps


def topk_like_jax(scores, k=TOPK, pad=16):
    """Top-k per row matching jax.lax.top_k ordering (desc value, then index)."""
    n = scores.shape[-1]
    kp = min(n, k + pad)
    part = np.argpartition(-scores, kp - 1, axis=-1)[..., :kp]
    part = np.sort(part, axis=-1)  # ascending index order -> stable tie-break
    vals = np.take_along_axis(scores, part, axis=-1)
    order = np.argsort(-vals, axis=-1, kind="stable")[..., :k]
    return np.take_along_axis(part, order, axis=-1).astype(np.int32)


def kernel(x, q_resid, freqs, wq_b, wk, w_weights, kn_gamma, kn_beta):
    xT, qrT, wqT, wkwT, cos, sin = host_prep(x, q_resid, freqs, wq_b, wk, w_weights)
    nc = _get_nc(N_CORES)
    in_maps = make_in_maps(xT, qrT, wqT, wkwT, cos, sin, N_CORES)
    res = run_bass_kernel_spmd(nc, in_maps, core_ids=list(range(N_CORES)))
    sc = np.concatenate([res.results[c]["scores"] for c in range(N_CORES)], axis=0)
    idx = topk_like_jax(sc)
    return idx.reshape(B, S, TOPK)
